# revision 12
# baseline (speedup 1.0000x reference)
"""Trainium2 Bass kernel for nn_CIN (xDeepFM compressed-interaction network).

Math: each CIN layer computes, per sample b and feature-dim d (a "column"
n=(b,d)):  y[o] = sum_{h,m} W[o,h,m] * a[h] * b[m]  — a bilinear form.

We avoid materializing the outer-product tensor z[h*m, n] (which needs slow
cross-partition broadcasts) by polarization:  a*b = ((a+b)^2 - a^2 - b^2)/2.
Each layer becomes:  s = V @ t   (pair sums, TensorE)
                     q = s*s     (elementwise square)
                     y = C @ q + G @ t^2   (TensorE, PSUM-accumulated)
with V a 0/1 pair-selection matrix and C,G folded from W host-side (exact).

Layer 0 uses the symmetric fold (741 unordered pairs of 39 features);
layer 1 uses all 64*39=2496 (nh,x) pairs.  Everything on-device is fp16
(inputs/weights) with fp32 PSUM accumulation.

Engine balance (per 512-column tile; 13 wide square-pairs total):
  PE   : 54 matmuls (V0 6, C0 6, G0 1, V1 20, C1 20, G1 1)  ~= 11.5us
  ACT  : 3 relu(+bias) + 8 wide squares (PSUM->SBUF)         ~= 10.1us
  DVE  : 2 sq pairs (copy+mul), 3 pair copies (for Pool),
         2 d-sum reduces                                     ~= 8.3us
  Pool : 3 pair muls + x^2 and nh^2 muls (all SBUF f16)      ~= 8.6us
(DVE TensorTensor cannot read PSUM twice — walrus verifier rejects it —
so non-ACT squares go copy-then-multiply; Pool has no PSUM port at all.)
V outputs are written pairwise into [128,1024] two-bank PSUM tiles so each
square instruction covers two chunks (amortizes the fixed access latency).
PSUM budget: 3x2 (V pairs) + 1 (y0) + 1 (y1) = 8 banks.

Sharding: pure data parallel — batch 4096 split as 512 per NeuronCore
across 8 cores; weights replicated.
"""

import numpy as np

B, F, D = 4096, 39, 16
L0, L1 = 128, 128
H1 = L0 // 2                      # 64 hidden maps feed layer 1
NCORES = 8
BL = B // NCORES                  # 512 samples per core
NCOL = BL * D                     # 8192 columns per core
NT = 512                          # columns per tile
NTILES = NCOL // NT               # 16
NB = NT // D                      # samples per tile (32)

K0 = F * (F - 1) // 2             # 741 layer-0 pairs
K1 = H1 * F                       # 2496 layer-1 pairs
NC0 = (K0 + 127) // 128           # 6 chunks (K0 padded to 768)
NC1 = (K1 + 127) // 128           # 20 chunks (K1 padded to 2560)
NP0 = NC0 // 2                    # 3 wide chunk-pairs
NP1 = NC1 // 2                    # 10 wide chunk-pairs
T1 = 128                          # t rows: [x 0:39 | zeros 39:64 | nh 64:128]
NH0 = 64                          # nh base partition in t


def _host_weights(W0, b0, W1, b1):
    """Fold W0/W1 into the square-trick operands (all exact, fp32)."""
    W0 = np.asarray(W0, np.float32)
    W1 = np.asarray(W1, np.float32)
    S0 = W0.reshape(L0, F, F)
    S0 = (S0 + S0.transpose(0, 2, 1)) / 2
    iu = np.triu_indices(F, 1)                       # 741 (h<m) pairs
    V0 = np.zeros((128 * NC0, F), np.float32)
    V0[np.arange(K0), iu[0]] = 1
    V0[np.arange(K0), iu[1]] = 1
    C0 = np.zeros((L0, 128 * NC0), np.float32)
    C0[:, :K0] = S0[:, iu[0], iu[1]]
    rowsum = S0.sum(2)
    G0 = np.einsum('ohh->oh', S0) * 2 - rowsum       # S[h,h] - sum_{m!=h} S[h,m]

    B1 = W1.reshape(L1, H1, F)
    hh, mm = np.meshgrid(np.arange(H1), np.arange(F), indexing='ij')
    hh, mm = hh.ravel(), mm.ravel()                  # 2496 pairs, h-major
    V1 = np.zeros((128 * NC1, T1), np.float32)
    V1[np.arange(K1), mm] = 1                        # x part at rows 0:39
    V1[np.arange(K1), NH0 + hh] = 1                  # nh part at rows 64:128
    C1 = np.zeros((L1, 128 * NC1), np.float32)
    C1[:, :K1] = B1[:, hh, mm] / 2
    G1 = np.zeros((L1, T1), np.float32)
    G1[:, :F] = -B1.sum(1) / 2                       # coeff on x^2
    G1[:, NH0:] = -B1.sum(2) / 2                     # coeff on nh^2

    def blockT(C):
        # [128, K] -> per-128-column-block transpose: lhsT[k, o] = C[o, base+k]
        L, K = C.shape
        return np.ascontiguousarray(
            C.reshape(L, K // 128, 128).transpose(2, 1, 0).reshape(128, -1)
        )

    return {
        "V0T": V0.T.astype(np.float16),              # [39, 768]
        "V1T": V1.T.astype(np.float16),              # [128, 2560]
        "C0T": blockT(C0).astype(np.float16),        # [128, 768]   (lhsT chunks)
        "C1T": blockT(C1).astype(np.float16),        # [128, 2560]
        "G0T": G0.T.astype(np.float16),              # [39, 128]
        "G1T": G1.T.astype(np.float16),              # [128, 128]
        "b0": np.asarray(b0, np.float32).reshape(L0, 1),
        "b1": np.asarray(b1, np.float32).reshape(L1, 1),
    }


_NC_CACHE = {}


def _build_nc(repeat=1):
    key = ("nc", repeat)
    if key in _NC_CACHE:
        return _NC_CACHE[key]
    from contextlib import ExitStack
    import concourse.bacc as bacc
    import concourse.mybir as mybir
    import concourse.tile as tile

    f16 = mybir.dt.float16
    f32 = mybir.dt.float32

    nc = bacc.Bacc("TRN2", target_bir_lowering=False, debug=False)

    xT_d = nc.dram_tensor("xT", [F, NCOL], f16, kind="ExternalInput")
    V0T_d = nc.dram_tensor("V0T", [F, 128 * NC0], f16, kind="ExternalInput")
    V1T_d = nc.dram_tensor("V1T", [T1, 128 * NC1], f16, kind="ExternalInput")
    C0T_d = nc.dram_tensor("C0T", [128, 128 * NC0], f16, kind="ExternalInput")
    C1T_d = nc.dram_tensor("C1T", [128, 128 * NC1], f16, kind="ExternalInput")
    G0T_d = nc.dram_tensor("G0T", [F, 128], f16, kind="ExternalInput")
    G1T_d = nc.dram_tensor("G1T", [T1, 128], f16, kind="ExternalInput")
    b0_d = nc.dram_tensor("b0", [L0, 1], f32, kind="ExternalInput")
    b1_d = nc.dram_tensor("b1", [L1, 1], f32, kind="ExternalInput")
    out_d = nc.dram_tensor("out", [L0 - H1 + L1, BL], f32, kind="ExternalOutput")

    Relu = mybir.ActivationFunctionType.Relu
    Square = mybir.ActivationFunctionType.Square

    with tile.TileContext(nc) as tc, ExitStack() as ctx:
        const = ctx.enter_context(tc.tile_pool(name="const", bufs=1))
        tp = ctx.enter_context(tc.tile_pool(name="tp", bufs=1))
        sqp = ctx.enter_context(tc.tile_pool(name="sqp", bufs=4))
        scrp = ctx.enter_context(tc.tile_pool(name="scrp", bufs=2))
        rp = ctx.enter_context(tc.tile_pool(name="rp", bufs=2))
        outp = ctx.enter_context(tc.tile_pool(name="outp", bufs=1))
        sps = ctx.enter_context(tc.tile_pool(name="sps", bufs=3, space="PSUM"))
        yps0 = ctx.enter_context(tc.tile_pool(name="yps0", bufs=1, space="PSUM"))
        yps1 = ctx.enter_context(tc.tile_pool(name="yps1", bufs=1, space="PSUM"))

        # resident weights
        V0T = const.tile([F, 128 * NC0], f16)
        V1T = const.tile([T1, 128 * NC1], f16)
        C0T = const.tile([128, 128 * NC0], f16)
        C1T = const.tile([128, 128 * NC1], f16)
        G0T = const.tile([F, 128], f16)
        G1T = const.tile([T1, 128], f16)
        b0t = const.tile([L0, 1], f32)
        b1t = const.tile([L1, 1], f32)
        for dst, src in ((V0T, V0T_d), (V1T, V1T_d), (C0T, C0T_d),
                         (C1T, C1T_d), (G0T, G0T_d), (G1T, G1T_d),
                         (b0t, b0_d), (b1t, b1_d)):
            nc.sync.dma_start(out=dst[:], in_=src.ap())

        # persistent double-buffered t = [x; 0; nh] and t2 = [x^2; 0; nh^2]
        t_bufs = [tp.tile([T1, NT], f16, name=f"t{i}", tag=f"t{i}")
                  for i in range(2)]
        t2_bufs = [tp.tile([T1, NT], f16, name=f"t2_{i}", tag=f"t2_{i}")
                   for i in range(2)]
        for tt in (*t_bufs, *t2_bufs):
            nc.vector.memset(tt[32:NH0, :], 0.0)     # one-time zero padding

        out0 = outp.tile([H1, BL], f32)
        out1 = outp.tile([L1, BL], f32)

        # Wide square of a [128, 2*NT] PSUM pair, dispatched per engine plan:
        #   'A': ACT square straight from PSUM
        #   'D': DVE copy to SBUF f16, DVE multiply
        #   'P': DVE copy to SBUF f16, Pool multiply
        def square(dst, src, how, scratch):
            if how == 'A':
                nc.scalar.square(dst, src)
            else:
                nc.vector.tensor_copy(scratch[:], src)
                eng = nc.vector if how == 'D' else nc.gpsimd
                eng.tensor_mul(dst, scratch[:], scratch[:])

        for it, nt in enumerate(
                [nt for _ in range(repeat) for nt in range(NTILES)]):
            csl = slice(nt * NT, (nt + 1) * NT)
            t = t_bufs[it % 2]
            t2 = t2_bufs[it % 2]
            nc.sync.dma_start(out=t[0:F, :], in_=xT_d.ap()[:, csl])
            # x^2 (rows 39:64 of t are zero, so the aligned 0:64 range is safe)
            nc.gpsimd.tensor_mul(t2[0:NH0, :], t[0:NH0, :], t[0:NH0, :])

            # ---- layer 0: s0 = V0 @ x ; q0 = s0^2 (3 wide pairs) ----
            PLAN0 = "APA"
            sq0 = []
            for p in range(NP0):
                ps = sps.tile([128, 2 * NT], f32)
                for h in range(2):
                    i = 2 * p + h
                    nc.tensor.matmul(ps[:, h * NT:(h + 1) * NT],
                                     V0T[:, i * 128:(i + 1) * 128],
                                     t[0:F, :], start=True, stop=True)
                sq = sqp.tile([128, 2 * NT], f16)
                scr = (scrp.tile([128, 2 * NT], f16, name="scr0") if PLAN0[p] != "A" else None)
                square(sq[:], ps[:], PLAN0[p], scr)
                sq0.append(sq)

            # ---- y0 = C0 @ q0 + G0 @ x^2 ----
            y0 = yps0.tile([L0, NT], f32)
            for i in range(NC0):
                nc.tensor.matmul(y0[:], C0T[:, i * 128:(i + 1) * 128],
                                 sq0[i // 2][:, (i % 2) * NT:(i % 2 + 1) * NT],
                                 start=(i == 0), stop=False)
            nc.tensor.matmul(y0[:], G0T[:], t2[0:F, :], start=False, stop=True)

            # relu + split: nh feeds layer 1, r0 is direct-connect
            nc.scalar.activation(t[NH0:T1, :], y0[0:H1, :], Relu, bias=b0t[0:H1])
            r0 = rp.tile([H1, NT], f32, tag="r0")
            nc.scalar.activation(r0[:], y0[H1:L0, :], Relu, bias=b0t[H1:L0])
            nc.gpsimd.tensor_mul(t2[NH0:T1, :], t[NH0:T1, :], t[NH0:T1, :])  # nh^2

            # ---- layer 1: s1 = V1 @ [x; nh] ; q1 = s1^2 (10 wide pairs) ----
            PLAN1 = "ADAPADAPAA"
            sq1 = []
            for p in range(NP1):
                ps = sps.tile([128, 2 * NT], f32)
                for h in range(2):
                    i = 2 * p + h
                    nc.tensor.matmul(ps[:, h * NT:(h + 1) * NT],
                                     V1T[:, i * 128:(i + 1) * 128],
                                     t[:], start=True, stop=True)
                sq = sqp.tile([128, 2 * NT], f16)
                scr = (scrp.tile([128, 2 * NT], f16, name="scr1") if PLAN1[p] != "A" else None)
                square(sq[:], ps[:], PLAN1[p], scr)
                sq1.append(sq)

            # ---- y1 = C1 @ q1 + G1 @ t^2 ----
            y1 = yps1.tile([L1, NT], f32)
            for i in range(NC1):
                nc.tensor.matmul(y1[:], C1T[:, i * 128:(i + 1) * 128],
                                 sq1[i // 2][:, (i % 2) * NT:(i % 2 + 1) * NT],
                                 start=(i == 0), stop=False)
            nc.tensor.matmul(y1[:], G1T[:], t2[:], start=False, stop=True)

            r1 = rp.tile([L1, NT], f32, tag="r1")
            nc.scalar.activation(r1[:], y1[:], Relu, bias=b1t[:])

            # ---- sum over d (innermost 16 of each column group) ----
            bsl = slice(nt * NB, (nt + 1) * NB)
            nc.vector.tensor_reduce(
                out0[:, bsl], r0[:].rearrange("p (b d) -> p b d", d=D),
                axis=mybir.AxisListType.X, op=mybir.AluOpType.add)
            nc.vector.tensor_reduce(
                out1[:, bsl], r1[:].rearrange("p (b d) -> p b d", d=D),
                axis=mybir.AxisListType.X, op=mybir.AluOpType.add)

        nc.sync.dma_start(out=out_d.ap()[0:H1, :], in_=out0[:])
        nc.sync.dma_start(out=out_d.ap()[H1:, :], in_=out1[:])

    nc.compile()
    _NC_CACHE[key] = nc
    return nc


def _run(inputs, trace=False):
    from concourse.bass_utils import run_bass_kernel_spmd

    x = np.asarray(inputs["x"], np.float32)
    w = _host_weights(inputs["W0"], inputs["b0"], inputs["W1"], inputs["b1"])
    nc = _build_nc()

    in_maps = []
    for c in range(NCORES):
        xs = x[c * BL:(c + 1) * BL]                          # [512, 39, 16]
        xT = np.ascontiguousarray(
            xs.transpose(1, 0, 2).reshape(F, NCOL)).astype(np.float16)
        m = {"xT": xT}
        m.update(w)
        in_maps.append(m)

    res = run_bass_kernel_spmd(nc, in_maps, core_ids=list(range(NCORES)),
                               trace=trace)
    out = np.empty((B, L0 - H1 + L1), np.float32)
    for c in range(NCORES):
        out[c * BL:(c + 1) * BL] = res.results[c]["out"].T
    return out, res


def kernel(**inputs):
    out, _ = _run(inputs)
    return out
